# revision 6
# baseline (speedup 1.0000x reference)
"""GAT-style bipartite graph attention layer (nn_BiGraphContrastLayer) on 8 trn2 cores.

Strategy (dst-sharded SPMD, one shared program):
  - Every core computes zel = x @ [W | W@Al | W@Ar] for all N nodes (replicated;
    bf16 matmul, fp32 accum) and writes a per-node row table
    zel_tab[n] = [z(512) | el(8) | er(8) | pad] (bf16, 640 elems = 1280B) to DRAM.
  - Each core owns 1250 dst nodes.  Their incoming edges (+ self loops), sorted
    by dst and grouped into 10 dst tiles of 128, are gathered per edge from
    zel_tab via SWDGE dma_gather (src row: 1280B; dst el/er tail: 256B).
  - v = exp(leaky_relu(el_src + er_dst)) per edge/head; messages msg = v * z_src
    (DVE, per-head broadcast); per-dst-tile segment sums via one-hot selection
    matmuls on the PE accumulating in PSUM: out_tile = SelT.T @ msg and
    s_tile = SelT.T @ v.  Final: out/s + bias.
  No inter-core communication; host concatenates the 8 dst slices.
"""
import os

import numpy as np
import ml_dtypes

import concourse.bacc as bacc
import concourse.bass as bass
import concourse.mybir as mybir
import concourse.tile as tile

BF = ml_dtypes.bfloat16
F32 = np.float32

NS, ND, E, DIN, H, DH = 10000, 10000, 320000, 512, 8, 64
NEG = 0.2
NCORES = 8
DPC = ND // NCORES          # 1250 dst nodes per core
N = NS + ND
NPAD = 20480                # node count padded to 160 tiles of 128
ROW = 640                   # zel row elems: z(512) | el(8) | er(8) | pad(112)
NTILES = (DPC + 127) // 128  # 10 dst tiles per core
PANEL = 2048                # phase-1 node panel (16 subtiles of 128)


# ----------------------------------------------------------------- host prep
def _wrap_idx(idx):
    """dma_gather index layout: idx i -> [i % 16, i // 16], replicated 8x."""
    k = len(idx)
    w = np.zeros((16, k // 16), np.int16)
    w[np.arange(k) % 16, np.arange(k) // 16] = idx
    return np.tile(w, (8, 1))


def _host_prep(x_src, x_dst, edge_src, edge_dst, W, attn_l, attn_r, bias):
    x = np.concatenate([x_src, x_dst], 0).astype(F32)       # [N, 512]
    xT = np.zeros((DIN, NPAD), BF)
    xT[:, :N] = x.T
    Al = np.zeros((DIN, H), F32)
    Ar = np.zeros((DIN, H), F32)
    for h in range(H):
        Al[h * DH:(h + 1) * DH, h] = attn_l[h]
        Ar[h * DH:(h + 1) * DH, h] = attn_r[h]
    Wext = np.concatenate([W, W @ Al, W @ Ar], 1).astype(BF)  # [512, 528]
    bias_rep = np.tile(bias[None, :].astype(F32), (128, 1))   # [128, 512]

    # per-(core, dst tile) edge lists, sorted by local dst
    edge_src = edge_src.astype(np.int64)
    edge_dst = edge_dst.astype(np.int64)
    tlists = [[None] * NTILES for _ in range(NCORES)]
    kmax = 0
    for c in range(NCORES):
        d0 = c * DPC
        m = (edge_dst >= d0) & (edge_dst < d0 + DPC)
        es = np.concatenate([edge_src[m],
                             NS + d0 + np.arange(DPC, dtype=np.int64)])
        ed = np.concatenate([edge_dst[m] - d0, np.arange(DPC, dtype=np.int64)])
        order = np.argsort(ed, kind="stable")
        es, ed = es[order], ed[order]
        for t in range(NTILES):
            sel = (ed >= t * 128) & (ed < (t + 1) * 128)
            tlists[c][t] = (es[sel], ed[sel] - t * 128)
            kmax = max(kmax, int(sel.sum()))
    k_tile = ((kmax + 127) // 128) * 128
    nch = k_tile // 128

    per_core = []
    for c in range(NCORES):
        zidx = np.zeros((128, NTILES * k_tile // 16), np.int16)
        eidx = np.zeros((128, NTILES * k_tile // 16), np.int16)
        selT = np.zeros((128, NTILES * nch * 128), BF)
        for t in range(NTILES):
            es, edl = tlists[c][t]
            k = len(es)
            src = np.zeros(k_tile, np.int64)
            src[:k] = es
            erx = np.zeros(k_tile, np.int64)
            erx[:k] = NS + c * DPC + t * 128 + edl
            s16 = slice(t * k_tile // 16, (t + 1) * k_tile // 16)
            zidx[:, s16] = _wrap_idx(src)
            eidx[:, s16] = _wrap_idx(erx)
            dstl = np.full(k_tile, -1, np.int64)
            dstl[:k] = edl
            for ch in range(nch):
                dl = dstl[ch * 128:(ch + 1) * 128]
                sm = np.zeros((128, 128), F32)
                valid = dl >= 0
                sm[np.arange(128)[valid], dl[valid]] = 1.0
                j = (t * nch + ch) * 128
                selT[:, j:j + 128] = sm.astype(BF)
        per_core.append(dict(selT=selT, zidx=zidx, eidx=eidx))

    shared = dict(xT=xT, Wext=Wext, bias_rep=bias_rep)
    return shared, per_core, k_tile, nch


# ------------------------------------------------------------- bass program
def _build_nc(k_tile, nch):
    nc = bacc.Bacc("TRN2", target_bir_lowering=False, debug=False)
    dt = mybir.dt

    xT_d = nc.dram_tensor("xT", [DIN, NPAD], dt.bfloat16, kind="ExternalInput")
    W_d = nc.dram_tensor("Wext", [DIN, 528], dt.bfloat16, kind="ExternalInput")
    bias_d = nc.dram_tensor("bias_rep", [128, 512], dt.float32, kind="ExternalInput")
    selT_d = nc.dram_tensor("selT", [128, NTILES * nch * 128], dt.bfloat16,
                            kind="ExternalInput")
    zidx_d = nc.dram_tensor("zidx", [128, NTILES * k_tile // 16], dt.int16,
                            kind="ExternalInput")
    eidx_d = nc.dram_tensor("eidx", [128, NTILES * k_tile // 16], dt.int16,
                            kind="ExternalInput")
    out_d = nc.dram_tensor("out", [NTILES * 128, 512], dt.float32,
                           kind="ExternalOutput")
    zel_d = nc.dram_tensor("zel_tab", [NPAD, ROW], dt.bfloat16)

    with tile.TileContext(nc) as tc:
        # ---- constants resident in SBUF
        with tc.tile_pool(name="const", bufs=1) as cpool:
            wsb = cpool.tile([128, 4 * 528], dt.bfloat16)
            for k in range(4):
                nc.sync.dma_start(wsb[:, k * 528:(k + 1) * 528],
                                  W_d[k * 128:(k + 1) * 128, :])
            bias_sb = cpool.tile([128, 512], dt.float32)
            nc.sync.dma_start(bias_sb[:], bias_d[:])
            zidx_sb = cpool.tile([128, NTILES * k_tile // 16], dt.int16)
            nc.sync.dma_start(zidx_sb[:], zidx_d[:])
            eidx_sb = cpool.tile([128, NTILES * k_tile // 16], dt.int16)
            nc.sync.dma_start(eidx_sb[:], eidx_d[:])

            # ---- phase 1: zel_tab = [x@W | x@Wl | x@Wr] for all nodes
            with (
                tc.tile_pool(name="xp", bufs=2) as xpool,
                tc.tile_pool(name="zel", bufs=3) as zpool,
                tc.tile_pool(name="p1", bufs=2, space="PSUM") as p1pool,
                tc.tile_pool(name="p1b", bufs=2, space="PSUM") as p1bpool,
            ):
                for p in range(NPAD // PANEL):
                    xp = xpool.tile([128, 4 * PANEL], dt.bfloat16)
                    for k in range(4):
                        nc.sync.dma_start(
                            xp[:, k * PANEL:(k + 1) * PANEL],
                            xT_d[k * 128:(k + 1) * 128,
                                 p * PANEL:(p + 1) * PANEL])
                    for m in range(PANEL // 128):
                        zps = p1pool.tile([128, 512], dt.float32, space="PSUM")
                        lps = p1bpool.tile([128, 16], dt.float32, space="PSUM")
                        for k in range(4):
                            lhsT = xp[:, k * PANEL + m * 128:
                                      k * PANEL + (m + 1) * 128]
                            nc.tensor.matmul(zps[:], lhsT,
                                             wsb[:, k * 528:k * 528 + 512],
                                             start=(k == 0), stop=(k == 3))
                            nc.tensor.matmul(lps[:], lhsT,
                                             wsb[:, k * 528 + 512:(k + 1) * 528],
                                             start=(k == 0), stop=(k == 3))
                        zel_sb = zpool.tile([128, ROW], dt.bfloat16)
                        nc.vector.tensor_copy(zel_sb[:, 0:512], zps[:])
                        nc.vector.tensor_copy(zel_sb[:, 512:528], lps[:])
                        nc.gpsimd.memset(zel_sb[:, 528:ROW], 0)
                        row0 = (p * (PANEL // 128) + m) * 128
                        nc.sync.dma_start(zel_d[row0:row0 + 128, :], zel_sb[:])

            # all phase-1 zel_tab writes must land before gathers read it
            tc.strict_bb_all_engine_barrier()

            # ---- phase 2: per dst tile gather + attention + aggregation
            with (
                tc.tile_pool(name="zg", bufs=2) as zgpool,
                tc.tile_pool(name="erg", bufs=2) as ergpool,
                tc.tile_pool(name="sel", bufs=2) as selpool,
                tc.tile_pool(name="sc", bufs=3) as scpool,
                tc.tile_pool(name="eo", bufs=2) as eopool,
                tc.tile_pool(name="p2", bufs=2, space="PSUM") as p2pool,
                tc.tile_pool(name="p2b", bufs=2, space="PSUM") as p2bpool,
            ):
                for t in range(NTILES):
                    i16 = slice(t * k_tile // 16, (t + 1) * k_tile // 16)
                    zg = zgpool.tile([128, nch, ROW], dt.bfloat16)
                    nc.gpsimd.dma_gather(
                        zg[:], zel_d[:], zidx_sb[:, i16],
                        num_idxs=k_tile, num_idxs_reg=k_tile, elem_size=ROW,
                        single_packet=False)
                    erg = ergpool.tile([128, nch, 128], dt.bfloat16)
                    nc.gpsimd.dma_gather(
                        erg[:], zel_d[:, 512:640], eidx_sb[:, i16],
                        num_idxs=k_tile, num_idxs_reg=k_tile, elem_size=128,
                        elem_step=ROW, single_packet=False)
                    sel = selpool.tile([128, nch * 128], dt.bfloat16)
                    nc.sync.dma_start(
                        sel[:], selT_d[:, t * nch * 128:(t + 1) * nch * 128])

                    # v = exp(leaky_relu(el_src + er_dst))  [128, nch, 8] f32
                    lt = scpool.tile([128, nch, 8], dt.float32, tag="lt")
                    nc.vector.tensor_tensor(
                        lt[:], zg[:, :, 512:520], erg[:, :, 8:16],
                        op=mybir.AluOpType.add)
                    nc.vector.scalar_tensor_tensor(
                        lt[:], lt[:], NEG, lt[:],
                        op0=mybir.AluOpType.mult, op1=mybir.AluOpType.max)
                    vt = scpool.tile([128, nch, 8], dt.float32, tag="vt")
                    nc.scalar.activation(vt[:], lt[:],
                                         mybir.ActivationFunctionType.Exp)
                    vb = scpool.tile([128, nch, 8], dt.bfloat16, tag="vb")
                    nc.vector.tensor_copy(vb[:], vt[:])

                    # msg = v * z  (in place over the z part of zg)
                    z4 = zg[:, :, 0:512].rearrange("p c (h d) -> p c h d", d=DH)
                    nc.vector.tensor_tensor(
                        z4, z4, vb[:].to_broadcast([128, nch, 8, DH]),
                        op=mybir.AluOpType.mult)

                    # segment sums on the PE
                    po = p2pool.tile([128, 512], dt.float32, space="PSUM")
                    ps = p2bpool.tile([128, 8], dt.float32, space="PSUM")
                    for ch in range(nch):
                        sl = sel[:, ch * 128:(ch + 1) * 128]
                        nc.tensor.matmul(po[:], sl, zg[:, ch, 0:512],
                                         start=(ch == 0), stop=(ch == nch - 1))
                        nc.tensor.matmul(ps[:], sl, vb[:, ch, :],
                                         start=(ch == 0), stop=(ch == nch - 1))

                    # out = po / s + bias  (eps keeps pad rows finite: 1/eps * 0 = 0)
                    ssb = scpool.tile([128, 8], dt.float32, tag="ssb")
                    nc.vector.tensor_scalar_add(ssb[:], ps[:], 1e-30)
                    nc.vector.reciprocal(ssb[:], ssb[:])
                    osb = eopool.tile([128, 512], dt.float32)
                    o4 = osb[:].rearrange("p (h d) -> p h d", d=DH)
                    nc.vector.tensor_tensor(
                        o4, po[:].rearrange("p (h d) -> p h d", d=DH),
                        ssb[:].to_broadcast([128, 8, DH]),
                        op=mybir.AluOpType.mult)
                    nc.vector.tensor_tensor(osb[:], osb[:], bias_sb[:],
                                            op=mybir.AluOpType.add)
                    nc.sync.dma_start(out_d[t * 128:(t + 1) * 128, :], osb[:])
    nc.compile()
    return nc


# ------------------------------------------------------------------- driver
def kernel(x_src, x_dst, edge_src, edge_dst, W, attn_l, attn_r, bias):
    shared, per_core, k_tile, nch = _host_prep(
        np.asarray(x_src), np.asarray(x_dst), np.asarray(edge_src),
        np.asarray(edge_dst), np.asarray(W), np.asarray(attn_l),
        np.asarray(attn_r), np.asarray(bias))

    nc = _build_nc(k_tile, nch)

    in_maps = []
    for c in range(NCORES):
        m = dict(xT=shared["xT"], Wext=shared["Wext"],
                 bias_rep=shared["bias_rep"], **per_core[c])
        in_maps.append({"xT": m["xT"], "Wext": m["Wext"],
                        "bias_rep": m["bias_rep"], "selT": m["selT"],
                        "zidx": m["zidx"], "eidx": m["eidx"]})

    if os.environ.get("KERNEL_SIM"):
        from concourse.bass_interp import CoreSim
        sim = CoreSim(nc, trace=False)
        for name, arr in in_maps[int(os.environ.get("KERNEL_SIM_CORE", "0"))].items():
            sim.tensor(name)[:] = arr
        sim.simulate()
        out = np.array(sim.tensor("out"))
        return np.concatenate([out[:DPC]] * NCORES, 0)  # core-0 slice only

    from concourse.bass_utils import run_bass_kernel_spmd
    res = run_bass_kernel_spmd(nc, in_maps, core_ids=list(range(NCORES)),
                               trace=bool(os.environ.get("KERNEL_TRACE")))
    global LAST_RESULTS
    LAST_RESULTS = res
    return np.concatenate([r["out"][:DPC] for r in res.results], 0)


LAST_RESULTS = None


# revision 14
# speedup vs baseline: 1.4048x; 1.4048x over previous
"""GAT-style bipartite graph attention layer (nn_BiGraphContrastLayer) on 8 trn2 cores.

Strategy (dst-sharded SPMD, one shared program):
  - Every core computes zel = x @ [W | W@Al | W@Ar] for all N nodes (replicated;
    bf16 matmul, fp32 accum) and writes a per-node row table
    zel_tab[n] = [z(512) | el(8) | er(8) | pad] (bf16, 640 elems = 1280B) to DRAM.
  - Each core owns 1250 dst nodes.  Their incoming edges (+ self loops), sorted
    by dst and grouped into 10 dst tiles of 128, are gathered per edge from
    zel_tab via SWDGE dma_gather (src row: 1280B; dst el/er tail: 256B).
  - v = exp(leaky_relu(el_src + er_dst)) per edge/head; messages msg = v * z_src
    (DVE, per-head broadcast); per-dst-tile segment sums via one-hot selection
    matmuls on the PE accumulating in PSUM: out_tile = SelT.T @ msg and
    s_tile = SelT.T @ v.  Final: out/s + bias.
  No inter-core communication; host concatenates the 8 dst slices.
"""
import os

import numpy as np
import ml_dtypes

import concourse.bacc as bacc
import concourse.bass as bass
import concourse.mybir as mybir
import concourse.tile as tile

BF = ml_dtypes.bfloat16
F32 = np.float32

NS, ND, E, DIN, H, DH = 10000, 10000, 320000, 512, 8, 64
NEG = 0.2
NCORES = 8
DPC = ND // NCORES          # 1250 dst nodes per core
N = NS + ND
NPAD = 20480                # node count padded to 160 tiles of 128
ROW = 640                   # zel row elems: z(512) | el(8) | er(8) | pad(112)
NTILES = (DPC + 127) // 128  # 10 dst tiles per core
PANEL = 2048                # phase-1 node panel (16 subtiles of 128)


# ----------------------------------------------------------------- host prep
def _wrap_idx(idx):
    """dma_gather index layout: idx i -> [i % 16, i // 16], replicated 8x."""
    k = len(idx)
    w = np.zeros((16, k // 16), np.int16)
    w[np.arange(k) % 16, np.arange(k) // 16] = idx
    return np.tile(w, (8, 1))


def _host_prep(x_src, x_dst, edge_src, edge_dst, W, attn_l, attn_r, bias):
    x = np.concatenate([x_src, x_dst], 0).astype(F32)       # [N, 512]
    xT = np.zeros((DIN, NPAD), BF)
    xT[:, :N] = x.T
    Al = np.zeros((DIN, H), F32)
    Ar = np.zeros((DIN, H), F32)
    for h in range(H):
        Al[h * DH:(h + 1) * DH, h] = attn_l[h]
        Ar[h * DH:(h + 1) * DH, h] = attn_r[h]
    Wext = np.concatenate([W, W @ Al, W @ Ar], 1).astype(BF)  # [512, 528]
    bias_rep = np.tile(bias[None, :].astype(F32), (128, 1))   # [128, 512]

    # per-(core, dst tile) edge lists, sorted by local dst
    edge_src = edge_src.astype(np.int64)
    edge_dst = edge_dst.astype(np.int64)
    tlists = [[None] * NTILES for _ in range(NCORES)]
    kmax = 0
    for c in range(NCORES):
        d0 = c * DPC
        m = (edge_dst >= d0) & (edge_dst < d0 + DPC)
        es = np.concatenate([edge_src[m],
                             NS + d0 + np.arange(DPC, dtype=np.int64)])
        ed = np.concatenate([edge_dst[m] - d0, np.arange(DPC, dtype=np.int64)])
        order = np.argsort(ed, kind="stable")
        es, ed = es[order], ed[order]
        for t in range(NTILES):
            sel = (ed >= t * 128) & (ed < (t + 1) * 128)
            tlists[c][t] = (es[sel], ed[sel] - t * 128)
            kmax = max(kmax, int(sel.sum()))
    k_tile = ((kmax + 127) // 128) * 128
    nch = k_tile // 128

    per_core = []
    for c in range(NCORES):
        zidx = np.zeros((128, NTILES * k_tile // 16), np.int16)
        selT = np.zeros((128, NTILES * nch * 128), BF)
        selD = np.zeros((128, NTILES * nch * 128), BF)
        for t in range(NTILES):
            es, edl = tlists[c][t]
            k = len(es)
            src = np.zeros(k_tile, np.int64)
            src[:k] = es
            s16 = slice(t * k_tile // 16, (t + 1) * k_tile // 16)
            zidx[:, s16] = _wrap_idx(src)
            dstl = np.full(k_tile, -1, np.int64)
            dstl[:k] = edl
            for ch in range(nch):
                dl = dstl[ch * 128:(ch + 1) * 128]
                sm = np.zeros((128, 128), F32)
                valid = dl >= 0
                sm[np.arange(128)[valid], dl[valid]] = 1.0
                j = (t * nch + ch) * 128
                selT[:, j:j + 128] = sm.astype(BF)
                selD[:, j:j + 128] = sm.T.astype(BF)
        eridx = _wrap_idx(NS + c * DPC + np.arange(NTILES * 128, dtype=np.int64))
        per_core.append(dict(selT=selT, selD=selD, zidx=zidx, eridx=eridx))

    shared = dict(xT=xT, Wext=Wext, bias_rep=bias_rep)
    return shared, per_core, k_tile, nch


# ------------------------------------------------------------- bass program
def _build_nc(k_tile, nch):
    nc = bacc.Bacc("TRN2", target_bir_lowering=False, debug=False)
    dt = mybir.dt

    xT_d = nc.dram_tensor("xT", [DIN, NPAD], dt.bfloat16, kind="ExternalInput")
    W_d = nc.dram_tensor("Wext", [DIN, 528], dt.bfloat16, kind="ExternalInput")
    bias_d = nc.dram_tensor("bias_rep", [128, 512], dt.float32, kind="ExternalInput")
    selT_d = nc.dram_tensor("selT", [128, NTILES * nch * 128], dt.bfloat16,
                            kind="ExternalInput")
    selD_d = nc.dram_tensor("selD", [128, NTILES * nch * 128], dt.bfloat16,
                            kind="ExternalInput")
    zidx_d = nc.dram_tensor("zidx", [128, NTILES * k_tile // 16], dt.int16,
                            kind="ExternalInput")
    eridx_d = nc.dram_tensor("eridx", [128, NTILES * 128 // 16], dt.int16,
                             kind="ExternalInput")
    out_d = nc.dram_tensor("out", [NTILES * 128, 512], dt.float32,
                           kind="ExternalOutput")
    zel_d = nc.dram_tensor("zel_tab", [NPAD, ROW], dt.bfloat16)

    with tile.TileContext(nc) as tc:
        # ---- constants resident in SBUF
        with tc.tile_pool(name="const", bufs=1) as cpool:
            wsb = cpool.tile([128, 4 * 528], dt.bfloat16)
            for k in range(4):
                nc.sync.dma_start(wsb[:, k * 528:(k + 1) * 528],
                                  W_d[k * 128:(k + 1) * 128, :])
            bias_sb = cpool.tile([128, 512], dt.float32)
            nc.sync.dma_start(bias_sb[:], bias_d[:])
            zidx_sb = cpool.tile([128, NTILES * k_tile // 16], dt.int16)
            nc.sync.dma_start(zidx_sb[:], zidx_d[:])
            eridx_sb = cpool.tile([128, NTILES * 128 // 16], dt.int16)
            nc.sync.dma_start(eridx_sb[:], eridx_d[:])

            # ---- phase 1: zel_tab = [x@W | x@Wl | x@Wr] for all nodes
            with (
                tc.tile_pool(name="xp", bufs=2) as xpool,
                tc.tile_pool(name="zel", bufs=3) as zpool,
                tc.tile_pool(name="p1", bufs=2, space="PSUM") as p1pool,
                tc.tile_pool(name="p1b", bufs=2, space="PSUM") as p1bpool,
            ):
                for p in range(NPAD // PANEL):
                    xp = xpool.tile([128, 4 * PANEL], dt.bfloat16)
                    for k in range(4):
                        nc.sync.dma_start(
                            xp[:, k * PANEL:(k + 1) * PANEL],
                            xT_d[k * 128:(k + 1) * 128,
                                 p * PANEL:(p + 1) * PANEL])
                    for m in range(PANEL // 128):
                        zps = p1pool.tile([128, 512], dt.float32, space="PSUM")
                        lps = p1bpool.tile([128, 16], dt.float32, space="PSUM")
                        for k in range(4):
                            lhsT = xp[:, k * PANEL + m * 128:
                                      k * PANEL + (m + 1) * 128]
                            nc.tensor.matmul(zps[:], lhsT,
                                             wsb[:, k * 528:k * 528 + 512],
                                             start=(k == 0), stop=(k == 3))
                            nc.tensor.matmul(lps[:], lhsT,
                                             wsb[:, k * 528 + 512:(k + 1) * 528],
                                             start=(k == 0), stop=(k == 3))
                        zel_sb = zpool.tile([128, ROW], dt.bfloat16)
                        nc.vector.tensor_copy(zel_sb[:, 0:512], zps[:])
                        nc.scalar.copy(zel_sb[:, 512:528], lps[:])
                        nc.gpsimd.memset(zel_sb[:, 528:ROW], 0)
                        row0 = (p * (PANEL // 128) + m) * 128
                        nc.sync.dma_start(zel_d[row0:row0 + 128, :], zel_sb[:])

            # all phase-1 zel_tab writes must land before gathers read it
            tc.strict_bb_all_engine_barrier()

            # ---- phase 2: per dst tile gather + attention + aggregation
            with (
                tc.tile_pool(name="zg", bufs=2) as zgpool,
                tc.tile_pool(name="era", bufs=1) as erapool,
                tc.tile_pool(name="sel", bufs=2) as selpool,
                tc.tile_pool(name="sc", bufs=3) as scpool,
                tc.tile_pool(name="eo", bufs=2) as eopool,
                tc.tile_pool(name="p2", bufs=2, space="PSUM") as p2pool,
                tc.tile_pool(name="p2b", bufs=2, space="PSUM") as p2bpool,
                tc.tile_pool(name="p2c", bufs=2, space="PSUM") as p2cpool,
            ):
                # er (and el) of this core's dst nodes: one small gather
                era = erapool.tile([128, NTILES, 128], dt.bfloat16)
                nc.gpsimd.dma_gather(
                    era[:], zel_d[:, 512:640], eridx_sb[:],
                    num_idxs=NTILES * 128, num_idxs_reg=NTILES * 128,
                    elem_size=128, elem_step=ROW, single_packet=False)

                for t in range(NTILES):
                    i16 = slice(t * k_tile // 16, (t + 1) * k_tile // 16)
                    zg = zgpool.tile([128, nch, ROW], dt.bfloat16)
                    nc.gpsimd.dma_gather(
                        zg[:], zel_d[:], zidx_sb[:, i16],
                        num_idxs=k_tile, num_idxs_reg=k_tile, elem_size=ROW,
                        single_packet=False)
                    sel = selpool.tile([128, nch * 128], dt.bfloat16)
                    nc.sync.dma_start(
                        sel[:], selT_d[:, t * nch * 128:(t + 1) * nch * 128])
                    seld = selpool.tile([128, nch * 128], dt.bfloat16,
                                        tag="seld")
                    nc.sync.dma_start(
                        seld[:], selD_d[:, t * nch * 128:(t + 1) * nch * 128])

                    # er_dst broadcast to edges via Sel matmul, then
                    # v = exp(leaky_relu(el_src + er_dst))  [128, nch, 8] f32
                    lt = scpool.tile([128, nch, 8], dt.float32, tag="lt")
                    for ch in range(nch):
                        pe_er = p2cpool.tile([128, 8], dt.float32, space="PSUM")
                        nc.tensor.matmul(pe_er[:],
                                         seld[:, ch * 128:(ch + 1) * 128],
                                         era[:, t, 8:16],
                                         start=True, stop=True)
                        nc.vector.tensor_tensor(
                            lt[:, ch, :], zg[:, ch, 512:520], pe_er[:],
                            op=mybir.AluOpType.add)
                    nc.vector.scalar_tensor_tensor(
                        lt[:], lt[:], NEG, lt[:],
                        op0=mybir.AluOpType.mult, op1=mybir.AluOpType.max)
                    vt = scpool.tile([128, nch, 8], dt.float32, tag="vt")
                    nc.scalar.activation(vt[:], lt[:],
                                         mybir.ActivationFunctionType.Exp)
                    vb = scpool.tile([128, nch, 8], dt.bfloat16, tag="vb")
                    nc.vector.tensor_copy(vb[:], vt[:])

                    # msg = v * z  (in place over the z part of zg)
                    z4 = zg[:, :, 0:512].rearrange("p c (h d) -> p c h d", d=DH)
                    nc.vector.tensor_tensor(
                        z4, z4, vb[:].to_broadcast([128, nch, 8, DH]),
                        op=mybir.AluOpType.mult)

                    # segment sums on the PE
                    po = p2pool.tile([128, 512], dt.float32, space="PSUM")
                    ps = p2bpool.tile([128, 8], dt.float32, space="PSUM")
                    for ch in range(nch):
                        sl = sel[:, ch * 128:(ch + 1) * 128]
                        nc.tensor.matmul(po[:], sl, zg[:, ch, 0:512],
                                         start=(ch == 0), stop=(ch == nch - 1))
                        nc.tensor.matmul(ps[:], sl, vb[:, ch, :],
                                         start=(ch == 0), stop=(ch == nch - 1))

                    # out = po / s + bias  (eps keeps pad rows finite: 1/eps * 0 = 0)
                    ssb = scpool.tile([128, 8], dt.float32, tag="ssb")
                    nc.vector.tensor_scalar_add(ssb[:], ps[:], 1e-30)
                    nc.vector.reciprocal(ssb[:], ssb[:])
                    osb = eopool.tile([128, 512], dt.float32)
                    o4 = osb[:].rearrange("p (h d) -> p h d", d=DH)
                    nc.vector.tensor_tensor(
                        o4, po[:].rearrange("p (h d) -> p h d", d=DH),
                        ssb[:].to_broadcast([128, 8, DH]),
                        op=mybir.AluOpType.mult)
                    nc.vector.tensor_tensor(osb[:], osb[:], bias_sb[:],
                                            op=mybir.AluOpType.add)
                    nc.sync.dma_start(out_d[t * 128:(t + 1) * 128, :], osb[:])
    nc.compile()
    return nc


# ------------------------------------------------------------------- driver
def kernel(x_src, x_dst, edge_src, edge_dst, W, attn_l, attn_r, bias):
    shared, per_core, k_tile, nch = _host_prep(
        np.asarray(x_src), np.asarray(x_dst), np.asarray(edge_src),
        np.asarray(edge_dst), np.asarray(W), np.asarray(attn_l),
        np.asarray(attn_r), np.asarray(bias))

    nc = _build_nc(k_tile, nch)

    in_maps = []
    for c in range(NCORES):
        in_maps.append({"xT": shared["xT"], "Wext": shared["Wext"],
                        "bias_rep": shared["bias_rep"],
                        "selT": per_core[c]["selT"],
                        "selD": per_core[c]["selD"],
                        "zidx": per_core[c]["zidx"],
                        "eridx": per_core[c]["eridx"]})

    if os.environ.get("KERNEL_SIM"):
        from concourse.bass_interp import CoreSim
        sim = CoreSim(nc, trace=False)
        for name, arr in in_maps[int(os.environ.get("KERNEL_SIM_CORE", "0"))].items():
            sim.tensor(name)[:] = arr
        sim.simulate()
        out = np.array(sim.tensor("out"))
        return np.concatenate([out[:DPC]] * NCORES, 0)  # core-0 slice only

    from concourse.bass_utils import run_bass_kernel_spmd
    res = run_bass_kernel_spmd(nc, in_maps, core_ids=list(range(NCORES)),
                               trace=bool(os.environ.get("KERNEL_TRACE")))
    global LAST_RESULTS
    LAST_RESULTS = res
    return np.concatenate([r["out"][:DPC] for r in res.results], 0)


LAST_RESULTS = None


# revision 15
# speedup vs baseline: 1.4293x; 1.0175x over previous
"""GAT-style bipartite graph attention layer (nn_BiGraphContrastLayer) on 8 trn2 cores.

Strategy (dst-sharded SPMD, one shared program):
  - Every core computes zel = x @ [W | W@Al | W@Ar] for all N nodes (replicated;
    bf16 matmul, fp32 accum) and writes a per-node row table
    zel_tab[n] = [z(512) | el(8) | er(8) | pad] (bf16, 640 elems = 1280B) to DRAM.
  - Each core owns 1250 dst nodes.  Their incoming edges (+ self loops), sorted
    by dst and grouped into 10 dst tiles of 128, are gathered per edge from
    zel_tab via SWDGE dma_gather (src row: 1280B; dst el/er tail: 256B).
  - v = exp(leaky_relu(el_src + er_dst)) per edge/head; messages msg = v * z_src
    (DVE, per-head broadcast); per-dst-tile segment sums via one-hot selection
    matmuls on the PE accumulating in PSUM: out_tile = SelT.T @ msg and
    s_tile = SelT.T @ v.  Final: out/s + bias.
  No inter-core communication; host concatenates the 8 dst slices.
"""
import os

import numpy as np
import ml_dtypes

import concourse.bacc as bacc
import concourse.bass as bass
import concourse.mybir as mybir
import concourse.tile as tile

BF = ml_dtypes.bfloat16
F32 = np.float32

NS, ND, E, DIN, H, DH = 10000, 10000, 320000, 512, 8, 64
NEG = 0.2
NCORES = 8
DPC = ND // NCORES          # 1250 dst nodes per core
N = NS + ND
NPAD = 20480                # node count padded to 160 tiles of 128
ROW = 640                   # zel row elems: z(512) | el(8) | er(8) | pad(112)
NTILES = (DPC + 127) // 128  # 10 dst tiles per core
PANEL = 2048                # phase-1 node panel (16 subtiles of 128)


# ----------------------------------------------------------------- host prep
def _wrap_idx(idx):
    """dma_gather index layout: idx i -> [i % 16, i // 16], replicated 8x."""
    k = len(idx)
    w = np.zeros((16, k // 16), np.int16)
    w[np.arange(k) % 16, np.arange(k) // 16] = idx
    return np.tile(w, (8, 1))


def _host_prep(x_src, x_dst, edge_src, edge_dst, W, attn_l, attn_r, bias):
    x = np.concatenate([x_src, x_dst], 0).astype(F32)       # [N, 512]
    xT = np.zeros((DIN, NPAD), BF)
    xT[:, :N] = x.T
    Al = np.zeros((DIN, H), F32)
    Ar = np.zeros((DIN, H), F32)
    for h in range(H):
        Al[h * DH:(h + 1) * DH, h] = attn_l[h]
        Ar[h * DH:(h + 1) * DH, h] = attn_r[h]
    Wext = np.concatenate([W, W @ Al, W @ Ar], 1).astype(BF)  # [512, 528]
    bias_rep = np.tile(bias[None, :].astype(F32), (128, 1))   # [128, 512]

    # per-(core, dst tile) edge lists, sorted by local dst
    edge_src = edge_src.astype(np.int64)
    edge_dst = edge_dst.astype(np.int64)
    tlists = [[None] * NTILES for _ in range(NCORES)]
    kmax = 0
    for c in range(NCORES):
        d0 = c * DPC
        m = (edge_dst >= d0) & (edge_dst < d0 + DPC)
        es = np.concatenate([edge_src[m],
                             NS + d0 + np.arange(DPC, dtype=np.int64)])
        ed = np.concatenate([edge_dst[m] - d0, np.arange(DPC, dtype=np.int64)])
        order = np.argsort(ed, kind="stable")
        es, ed = es[order], ed[order]
        for t in range(NTILES):
            sel = (ed >= t * 128) & (ed < (t + 1) * 128)
            tlists[c][t] = (es[sel], ed[sel] - t * 128)
            kmax = max(kmax, int(sel.sum()))
    k_tile = ((kmax + 127) // 128) * 128
    nch = k_tile // 128

    per_core = []
    for c in range(NCORES):
        zidx = np.zeros((128, NTILES * k_tile // 16), np.int16)
        selT = np.zeros((128, NTILES * nch * 128), BF)
        selD = np.zeros((128, NTILES * nch * 128), BF)
        for t in range(NTILES):
            es, edl = tlists[c][t]
            k = len(es)
            src = np.zeros(k_tile, np.int64)
            src[:k] = es
            s16 = slice(t * k_tile // 16, (t + 1) * k_tile // 16)
            zidx[:, s16] = _wrap_idx(src)
            dstl = np.full(k_tile, -1, np.int64)
            dstl[:k] = edl
            for ch in range(nch):
                dl = dstl[ch * 128:(ch + 1) * 128]
                sm = np.zeros((128, 128), F32)
                valid = dl >= 0
                sm[np.arange(128)[valid], dl[valid]] = 1.0
                j = (t * nch + ch) * 128
                selT[:, j:j + 128] = sm.astype(BF)
                selD[:, j:j + 128] = sm.T.astype(BF)
        eridx = _wrap_idx(NS + c * DPC + np.arange(NTILES * 128, dtype=np.int64))
        per_core.append(dict(selT=selT, selD=selD, zidx=zidx, eridx=eridx))

    shared = dict(xT=xT, Wext=Wext, bias_rep=bias_rep)
    return shared, per_core, k_tile, nch


# ------------------------------------------------------------- bass program
def _build_nc(k_tile, nch):
    nc = bacc.Bacc("TRN2", target_bir_lowering=False, debug=False)
    dt = mybir.dt

    xT_d = nc.dram_tensor("xT", [DIN, NPAD], dt.bfloat16, kind="ExternalInput")
    W_d = nc.dram_tensor("Wext", [DIN, 528], dt.bfloat16, kind="ExternalInput")
    bias_d = nc.dram_tensor("bias_rep", [128, 512], dt.float32, kind="ExternalInput")
    selT_d = nc.dram_tensor("selT", [128, NTILES * nch * 128], dt.bfloat16,
                            kind="ExternalInput")
    selD_d = nc.dram_tensor("selD", [128, NTILES * nch * 128], dt.bfloat16,
                            kind="ExternalInput")
    zidx_d = nc.dram_tensor("zidx", [128, NTILES * k_tile // 16], dt.int16,
                            kind="ExternalInput")
    eridx_d = nc.dram_tensor("eridx", [128, NTILES * 128 // 16], dt.int16,
                             kind="ExternalInput")
    out_d = nc.dram_tensor("out", [NTILES * 128, 512], dt.float32,
                           kind="ExternalOutput")
    zel_d = nc.dram_tensor("zel_tab", [NPAD, ROW], dt.bfloat16)

    with tile.TileContext(nc) as tc:
        # ---- constants resident in SBUF
        with tc.tile_pool(name="const", bufs=1) as cpool:
            wsb = cpool.tile([128, 4 * 528], dt.bfloat16)
            for k in range(4):
                nc.sync.dma_start(wsb[:, k * 528:(k + 1) * 528],
                                  W_d[k * 128:(k + 1) * 128, :])
            bias_sb = cpool.tile([128, 512], dt.float32)
            nc.sync.dma_start(bias_sb[:], bias_d[:])
            zidx_sb = cpool.tile([128, NTILES * k_tile // 16], dt.int16)
            nc.sync.dma_start(zidx_sb[:], zidx_d[:])
            eridx_sb = cpool.tile([128, NTILES * 128 // 16], dt.int16)
            nc.sync.dma_start(eridx_sb[:], eridx_d[:])

            # ---- phase 1: zel_tab = [x@W | x@Wl | x@Wr] for all nodes
            with (
                tc.tile_pool(name="xp", bufs=2) as xpool,
                tc.tile_pool(name="zel", bufs=3) as zpool,
                tc.tile_pool(name="p1", bufs=2, space="PSUM") as p1pool,
                tc.tile_pool(name="p1b", bufs=2, space="PSUM") as p1bpool,
            ):
                for p in range(NPAD // PANEL):
                    xp = xpool.tile([128, 4 * PANEL], dt.bfloat16)
                    for k in range(4):
                        nc.sync.dma_start(
                            xp[:, k * PANEL:(k + 1) * PANEL],
                            xT_d[k * 128:(k + 1) * 128,
                                 p * PANEL:(p + 1) * PANEL])
                    for m in range(PANEL // 128):
                        zps = p1pool.tile([128, 512], dt.float32, space="PSUM")
                        lps = p1bpool.tile([128, 16], dt.float32, space="PSUM")
                        for k in range(4):
                            lhsT = xp[:, k * PANEL + m * 128:
                                      k * PANEL + (m + 1) * 128]
                            nc.tensor.matmul(zps[:], lhsT,
                                             wsb[:, k * 528:k * 528 + 512],
                                             start=(k == 0), stop=(k == 3))
                            nc.tensor.matmul(lps[:], lhsT,
                                             wsb[:, k * 528 + 512:(k + 1) * 528],
                                             start=(k == 0), stop=(k == 3))
                        zel_sb = zpool.tile([128, ROW], dt.bfloat16)
                        nc.vector.tensor_copy(zel_sb[:, 0:512], zps[:])
                        nc.vector.tensor_copy(zel_sb[:, 512:528], lps[:])
                        nc.gpsimd.memset(zel_sb[:, 528:ROW], 0)
                        row0 = (p * (PANEL // 128) + m) * 128
                        nc.sync.dma_start(zel_d[row0:row0 + 128, :], zel_sb[:])

            # all phase-1 zel_tab writes must land before gathers read it
            tc.strict_bb_all_engine_barrier()

            # ---- phase 2: per dst tile gather + attention + aggregation
            with (
                tc.tile_pool(name="zg", bufs=2) as zgpool,
                tc.tile_pool(name="era", bufs=1) as erapool,
                tc.tile_pool(name="sel", bufs=2) as selpool,
                tc.tile_pool(name="sc", bufs=3) as scpool,
                tc.tile_pool(name="eo", bufs=2) as eopool,
                tc.tile_pool(name="p2", bufs=2, space="PSUM") as p2pool,
                tc.tile_pool(name="p2b", bufs=2, space="PSUM") as p2bpool,
                tc.tile_pool(name="p2c", bufs=2, space="PSUM") as p2cpool,
            ):
                # er (and el) of this core's dst nodes: one small gather
                era = erapool.tile([128, NTILES, 128], dt.bfloat16)
                nc.gpsimd.dma_gather(
                    era[:], zel_d[:, 512:640], eridx_sb[:],
                    num_idxs=NTILES * 128, num_idxs_reg=NTILES * 128,
                    elem_size=128, elem_step=ROW, single_packet=False)

                for t in range(NTILES):
                    i16 = slice(t * k_tile // 16, (t + 1) * k_tile // 16)
                    zg = zgpool.tile([128, nch, ROW], dt.bfloat16)
                    nc.gpsimd.dma_gather(
                        zg[:], zel_d[:], zidx_sb[:, i16],
                        num_idxs=k_tile, num_idxs_reg=k_tile, elem_size=ROW,
                        single_packet=False)
                    sel = selpool.tile([128, nch * 128], dt.bfloat16)
                    nc.sync.dma_start(
                        sel[:], selT_d[:, t * nch * 128:(t + 1) * nch * 128])
                    seld = selpool.tile([128, nch * 128], dt.bfloat16,
                                        tag="seld")
                    nc.sync.dma_start(
                        seld[:], selD_d[:, t * nch * 128:(t + 1) * nch * 128])

                    # er_dst broadcast to edges via Sel matmul, then
                    # v = exp(leaky_relu(el_src + er_dst))  [128, nch, 8] f32
                    lt = scpool.tile([128, nch, 8], dt.float32, tag="lt")
                    for ch in range(nch):
                        pe_er = p2cpool.tile([128, 8], dt.float32, space="PSUM")
                        nc.tensor.matmul(pe_er[:],
                                         seld[:, ch * 128:(ch + 1) * 128],
                                         era[:, t, 8:16],
                                         start=True, stop=True)
                        nc.vector.tensor_tensor(
                            lt[:, ch, :], zg[:, ch, 512:520], pe_er[:],
                            op=mybir.AluOpType.add)
                    nc.vector.scalar_tensor_tensor(
                        lt[:], lt[:], NEG, lt[:],
                        op0=mybir.AluOpType.mult, op1=mybir.AluOpType.max)
                    vt = scpool.tile([128, nch, 8], dt.float32, tag="vt")
                    nc.scalar.activation(vt[:], lt[:],
                                         mybir.ActivationFunctionType.Exp)
                    vb = scpool.tile([128, nch, 8], dt.bfloat16, tag="vb")
                    nc.vector.tensor_copy(vb[:], vt[:])

                    # msg = v * z  (in place over the z part of zg)
                    z4 = zg[:, :, 0:512].rearrange("p c (h d) -> p c h d", d=DH)
                    nc.vector.tensor_tensor(
                        z4, z4, vb[:].to_broadcast([128, nch, 8, DH]),
                        op=mybir.AluOpType.mult)

                    # segment sums on the PE
                    po = p2pool.tile([128, 512], dt.float32, space="PSUM")
                    ps = p2bpool.tile([128, 8], dt.float32, space="PSUM")
                    for ch in range(nch):
                        sl = sel[:, ch * 128:(ch + 1) * 128]
                        nc.tensor.matmul(po[:], sl, zg[:, ch, 0:512],
                                         start=(ch == 0), stop=(ch == nch - 1))
                        nc.tensor.matmul(ps[:], sl, vb[:, ch, :],
                                         start=(ch == 0), stop=(ch == nch - 1))

                    # out = po / s + bias  (eps keeps pad rows finite: 1/eps * 0 = 0)
                    ssb = scpool.tile([128, 8], dt.float32, tag="ssb")
                    nc.vector.tensor_scalar_add(ssb[:], ps[:], 1e-30)
                    nc.vector.reciprocal(ssb[:], ssb[:])
                    osb = eopool.tile([128, 512], dt.float32)
                    o4 = osb[:].rearrange("p (h d) -> p h d", d=DH)
                    nc.vector.tensor_tensor(
                        o4, po[:].rearrange("p (h d) -> p h d", d=DH),
                        ssb[:].to_broadcast([128, 8, DH]),
                        op=mybir.AluOpType.mult)
                    nc.vector.tensor_tensor(osb[:], osb[:], bias_sb[:],
                                            op=mybir.AluOpType.add)
                    nc.sync.dma_start(out_d[t * 128:(t + 1) * 128, :], osb[:])
    nc.compile()
    return nc


# ------------------------------------------------------------------- driver
def kernel(x_src, x_dst, edge_src, edge_dst, W, attn_l, attn_r, bias):
    shared, per_core, k_tile, nch = _host_prep(
        np.asarray(x_src), np.asarray(x_dst), np.asarray(edge_src),
        np.asarray(edge_dst), np.asarray(W), np.asarray(attn_l),
        np.asarray(attn_r), np.asarray(bias))

    nc = _build_nc(k_tile, nch)

    in_maps = []
    for c in range(NCORES):
        in_maps.append({"xT": shared["xT"], "Wext": shared["Wext"],
                        "bias_rep": shared["bias_rep"],
                        "selT": per_core[c]["selT"],
                        "selD": per_core[c]["selD"],
                        "zidx": per_core[c]["zidx"],
                        "eridx": per_core[c]["eridx"]})

    if os.environ.get("KERNEL_SIM"):
        from concourse.bass_interp import CoreSim
        sim = CoreSim(nc, trace=False)
        for name, arr in in_maps[int(os.environ.get("KERNEL_SIM_CORE", "0"))].items():
            sim.tensor(name)[:] = arr
        sim.simulate()
        out = np.array(sim.tensor("out"))
        return np.concatenate([out[:DPC]] * NCORES, 0)  # core-0 slice only

    from concourse.bass_utils import run_bass_kernel_spmd
    res = run_bass_kernel_spmd(nc, in_maps, core_ids=list(range(NCORES)),
                               trace=bool(os.environ.get("KERNEL_TRACE")))
    global LAST_RESULTS
    LAST_RESULTS = res
    return np.concatenate([r["out"][:DPC] for r in res.results], 0)


LAST_RESULTS = None


# revision 16
# speedup vs baseline: 1.4996x; 1.0492x over previous
"""GAT-style bipartite graph attention layer (nn_BiGraphContrastLayer) on 8 trn2 cores.

Strategy (dst-sharded SPMD, one shared program):
  - Every core computes zel = x @ [W | W@Al | W@Ar] for all N nodes (replicated;
    bf16 matmul, fp32 accum) and writes a per-node row table
    zel_tab[n] = [z(512) | el(8) | er(8) | pad] (bf16, 640 elems = 1280B) to DRAM.
  - Each core owns 1250 dst nodes.  Their incoming edges (+ self loops), sorted
    by dst and grouped into 10 dst tiles of 128, are gathered per edge from
    zel_tab via SWDGE dma_gather (src row: 1280B; dst el/er tail: 256B).
  - v = exp(leaky_relu(el_src + er_dst)) per edge/head; messages msg = v * z_src
    (DVE, per-head broadcast); per-dst-tile segment sums via one-hot selection
    matmuls on the PE accumulating in PSUM: out_tile = SelT.T @ msg and
    s_tile = SelT.T @ v.  Final: out/s + bias.
  No inter-core communication; host concatenates the 8 dst slices.
"""
import os

import numpy as np
import ml_dtypes

import concourse.bacc as bacc
import concourse.bass as bass
import concourse.mybir as mybir
import concourse.tile as tile

BF = ml_dtypes.bfloat16
F32 = np.float32

NS, ND, E, DIN, H, DH = 10000, 10000, 320000, 512, 8, 64
NEG = 0.2
NCORES = 8
DPC = ND // NCORES          # 1250 dst nodes per core
N = NS + ND
NPAD = 20480                # node count padded to 160 tiles of 128
ROW = 640                   # zel row elems: z(512) | el(8) | er(8) | pad(112)
NTILES = (DPC + 127) // 128  # 10 dst tiles per core
PANEL = 2048                # phase-1 node panel (16 subtiles of 128)


# ----------------------------------------------------------------- host prep
def _wrap_idx(idx):
    """dma_gather index layout: idx i -> [i % 16, i // 16], replicated 8x."""
    k = len(idx)
    w = np.zeros((16, k // 16), np.int16)
    w[np.arange(k) % 16, np.arange(k) // 16] = idx
    return np.tile(w, (8, 1))


def _host_prep(x_src, x_dst, edge_src, edge_dst, W, attn_l, attn_r, bias):
    x = np.concatenate([x_src, x_dst], 0).astype(F32)       # [N, 512]
    xT = np.zeros((DIN, NPAD), BF)
    xT[:, :N] = x.T
    Al = np.zeros((DIN, H), F32)
    Ar = np.zeros((DIN, H), F32)
    for h in range(H):
        Al[h * DH:(h + 1) * DH, h] = attn_l[h]
        Ar[h * DH:(h + 1) * DH, h] = attn_r[h]
    Wext = np.concatenate([W, W @ Al, W @ Ar], 1).astype(BF)  # [512, 528]
    bias_rep = np.tile(bias[None, :].astype(F32), (128, 1))   # [128, 512]

    # per-(core, dst tile) edge lists, sorted by local dst
    edge_src = edge_src.astype(np.int64)
    edge_dst = edge_dst.astype(np.int64)
    tlists = [[None] * NTILES for _ in range(NCORES)]
    kmax = 0
    for c in range(NCORES):
        d0 = c * DPC
        m = (edge_dst >= d0) & (edge_dst < d0 + DPC)
        es = np.concatenate([edge_src[m],
                             NS + d0 + np.arange(DPC, dtype=np.int64)])
        ed = np.concatenate([edge_dst[m] - d0, np.arange(DPC, dtype=np.int64)])
        order = np.argsort(ed, kind="stable")
        es, ed = es[order], ed[order]
        for t in range(NTILES):
            sel = (ed >= t * 128) & (ed < (t + 1) * 128)
            tlists[c][t] = (es[sel], ed[sel] - t * 128)
            kmax = max(kmax, int(sel.sum()))
    k_tile = ((kmax + 127) // 128) * 128
    nch = k_tile // 128

    per_core = []
    for c in range(NCORES):
        zidx = np.zeros((128, NTILES * k_tile // 16), np.int16)
        selT = np.zeros((128, NTILES * nch * 128), BF)
        selD = np.zeros((128, NTILES * nch * 128), BF)
        for t in range(NTILES):
            es, edl = tlists[c][t]
            k = len(es)
            src = np.zeros(k_tile, np.int64)
            src[:k] = es
            s16 = slice(t * k_tile // 16, (t + 1) * k_tile // 16)
            zidx[:, s16] = _wrap_idx(src)
            dstl = np.full(k_tile, -1, np.int64)
            dstl[:k] = edl
            for ch in range(nch):
                dl = dstl[ch * 128:(ch + 1) * 128]
                sm = np.zeros((128, 128), F32)
                valid = dl >= 0
                sm[np.arange(128)[valid], dl[valid]] = 1.0
                j = (t * nch + ch) * 128
                selT[:, j:j + 128] = sm.astype(BF)
                selD[:, j:j + 128] = sm.T.astype(BF)
        eridx = _wrap_idx(NS + c * DPC + np.arange(NTILES * 128, dtype=np.int64))
        per_core.append(dict(selT=selT, selD=selD, zidx=zidx, eridx=eridx))

    shared = dict(xT=xT, Wext=Wext, bias_rep=bias_rep)
    return shared, per_core, k_tile, nch


# ------------------------------------------------------------- bass program
def _build_nc(k_tile, nch):
    nc = bacc.Bacc("TRN2", target_bir_lowering=False, debug=False)
    dt = mybir.dt

    xT_d = nc.dram_tensor("xT", [DIN, NPAD], dt.bfloat16, kind="ExternalInput")
    W_d = nc.dram_tensor("Wext", [DIN, 528], dt.bfloat16, kind="ExternalInput")
    bias_d = nc.dram_tensor("bias_rep", [128, 512], dt.float32, kind="ExternalInput")
    selT_d = nc.dram_tensor("selT", [128, NTILES * nch * 128], dt.bfloat16,
                            kind="ExternalInput")
    selD_d = nc.dram_tensor("selD", [128, NTILES * nch * 128], dt.bfloat16,
                            kind="ExternalInput")
    zidx_d = nc.dram_tensor("zidx", [128, NTILES * k_tile // 16], dt.int16,
                            kind="ExternalInput")
    eridx_d = nc.dram_tensor("eridx", [128, NTILES * 128 // 16], dt.int16,
                             kind="ExternalInput")
    out_d = nc.dram_tensor("out", [NTILES * 128, 512], dt.float32,
                           kind="ExternalOutput")
    zel_d = nc.dram_tensor("zel_tab", [NPAD, ROW], dt.bfloat16)

    with tile.TileContext(nc) as tc:
        # ---- constants resident in SBUF
        with tc.tile_pool(name="const", bufs=1) as cpool:
            wsb = cpool.tile([128, 4 * 528], dt.bfloat16)
            for k in range(4):
                nc.sync.dma_start(wsb[:, k * 528:(k + 1) * 528],
                                  W_d[k * 128:(k + 1) * 128, :])
            bias_sb = cpool.tile([128, 512], dt.float32)
            nc.sync.dma_start(bias_sb[:], bias_d[:])
            zidx_sb = cpool.tile([128, NTILES * k_tile // 16], dt.int16)
            nc.sync.dma_start(zidx_sb[:], zidx_d[:])
            eridx_sb = cpool.tile([128, NTILES * 128 // 16], dt.int16)
            nc.sync.dma_start(eridx_sb[:], eridx_d[:])

            # ---- phase 1: zel_tab = [x@W | x@Wl | x@Wr] for all nodes
            with (
                tc.tile_pool(name="xp", bufs=2) as xpool,
                tc.tile_pool(name="zel", bufs=3) as zpool,
                tc.tile_pool(name="p1", bufs=2, space="PSUM") as p1pool,
                tc.tile_pool(name="p1b", bufs=2, space="PSUM") as p1bpool,
            ):
                for p in range(NPAD // PANEL):
                    xp = xpool.tile([128, 4 * PANEL], dt.bfloat16)
                    for k in range(4):
                        nc.sync.dma_start(
                            xp[:, k * PANEL:(k + 1) * PANEL],
                            xT_d[k * 128:(k + 1) * 128,
                                 p * PANEL:(p + 1) * PANEL])
                    for m in range(PANEL // 128):
                        zps = p1pool.tile([128, 512], dt.float32, space="PSUM")
                        lps = p1bpool.tile([128, 16], dt.float32, space="PSUM")
                        for k in range(4):
                            lhsT = xp[:, k * PANEL + m * 128:
                                      k * PANEL + (m + 1) * 128]
                            nc.tensor.matmul(zps[:], lhsT,
                                             wsb[:, k * 528:k * 528 + 512],
                                             start=(k == 0), stop=(k == 3))
                            nc.tensor.matmul(lps[:], lhsT,
                                             wsb[:, k * 528 + 512:(k + 1) * 528],
                                             start=(k == 0), stop=(k == 3))
                        zel_sb = zpool.tile([128, ROW], dt.bfloat16)
                        nc.vector.tensor_copy(zel_sb[:, 0:512], zps[:])
                        nc.vector.tensor_copy(zel_sb[:, 512:528], lps[:])
                        nc.gpsimd.memset(zel_sb[:, 528:ROW], 0)
                        row0 = (p * (PANEL // 128) + m) * 128
                        nc.sync.dma_start(zel_d[row0:row0 + 128, :], zel_sb[:])

            # all phase-1 zel_tab writes must land before gathers read it
            tc.strict_bb_all_engine_barrier()

            # ---- phase 2: per dst tile gather + attention + aggregation
            with (
                tc.tile_pool(name="zg", bufs=2) as zgpool,
                tc.tile_pool(name="era", bufs=1) as erapool,
                tc.tile_pool(name="sel", bufs=2) as selpool,
                tc.tile_pool(name="sc", bufs=3) as scpool,
                tc.tile_pool(name="eo", bufs=2) as eopool,
                tc.tile_pool(name="p2", bufs=2, space="PSUM") as p2pool,
                tc.tile_pool(name="p2b", bufs=2, space="PSUM") as p2bpool,
                tc.tile_pool(name="p2c", bufs=2, space="PSUM") as p2cpool,
            ):
                # er (and el) of this core's dst nodes: one small gather
                era = erapool.tile([128, NTILES, 128], dt.bfloat16)
                nc.gpsimd.dma_gather(
                    era[:], zel_d[:, 512:640], eridx_sb[:],
                    num_idxs=NTILES * 128, num_idxs_reg=NTILES * 128,
                    elem_size=128, elem_step=ROW, single_packet=False)

                for t in range(NTILES):
                    i16 = slice(t * k_tile // 16, (t + 1) * k_tile // 16)
                    zg = zgpool.tile([128, nch, ROW], dt.bfloat16)
                    nc.gpsimd.dma_gather(
                        zg[:], zel_d[:], zidx_sb[:, i16],
                        num_idxs=k_tile, num_idxs_reg=k_tile, elem_size=ROW,
                        single_packet=False)
                    sel = selpool.tile([128, nch * 128], dt.bfloat16)
                    nc.sync.dma_start(
                        sel[:], selT_d[:, t * nch * 128:(t + 1) * nch * 128])
                    seld = selpool.tile([128, nch * 128], dt.bfloat16,
                                        tag="seld")
                    nc.sync.dma_start(
                        seld[:], selD_d[:, t * nch * 128:(t + 1) * nch * 128])

                    # er_dst broadcast to edges via Sel matmuls — all chunks
                    # packed into one PSUM bank, then ONE logit add.
                    lt = scpool.tile([128, nch, 8], dt.float32, tag="lt")
                    pe_er = p2cpool.tile([128, nch, 8], dt.float32,
                                         space="PSUM")
                    for ch in range(nch):
                        nc.tensor.matmul(pe_er[:, ch, :],
                                         seld[:, ch * 128:(ch + 1) * 128],
                                         era[:, t, 8:16],
                                         start=True, stop=True,
                                         skip_group_check=True)
                    nc.vector.tensor_tensor(
                        lt[:], zg[:, :, 512:520], pe_er[:],
                        op=mybir.AluOpType.add)
                    nc.vector.scalar_tensor_tensor(
                        lt[:], lt[:], NEG, lt[:],
                        op0=mybir.AluOpType.mult, op1=mybir.AluOpType.max)
                    vt = scpool.tile([128, nch, 8], dt.float32, tag="vt")
                    nc.scalar.activation(vt[:], lt[:],
                                         mybir.ActivationFunctionType.Exp)
                    vb = scpool.tile([128, nch, 8], dt.bfloat16, tag="vb")
                    nc.vector.tensor_copy(vb[:], vt[:])

                    # msg = v * z  (in place over the z part of zg)
                    z4 = zg[:, :, 0:512].rearrange("p c (h d) -> p c h d", d=DH)
                    nc.vector.tensor_tensor(
                        z4, z4, vb[:].to_broadcast([128, nch, 8, DH]),
                        op=mybir.AluOpType.mult)

                    # segment sums on the PE
                    po = p2pool.tile([128, 512], dt.float32, space="PSUM")
                    ps = p2bpool.tile([128, 8], dt.float32, space="PSUM")
                    for ch in range(nch):
                        sl = sel[:, ch * 128:(ch + 1) * 128]
                        nc.tensor.matmul(po[:], sl, zg[:, ch, 0:512],
                                         start=(ch == 0), stop=(ch == nch - 1))
                        nc.tensor.matmul(ps[:], sl, vb[:, ch, :],
                                         start=(ch == 0), stop=(ch == nch - 1))

                    # out = po / s + bias  (eps keeps pad rows finite: 1/eps * 0 = 0)
                    ssb = scpool.tile([128, 8], dt.float32, tag="ssb")
                    nc.vector.tensor_scalar_add(ssb[:], ps[:], 1e-30)
                    nc.vector.reciprocal(ssb[:], ssb[:])
                    osb = eopool.tile([128, 512], dt.float32)
                    o4 = osb[:].rearrange("p (h d) -> p h d", d=DH)
                    nc.vector.tensor_tensor(
                        o4, po[:].rearrange("p (h d) -> p h d", d=DH),
                        ssb[:].to_broadcast([128, 8, DH]),
                        op=mybir.AluOpType.mult)
                    nc.vector.tensor_tensor(osb[:], osb[:], bias_sb[:],
                                            op=mybir.AluOpType.add)
                    nc.sync.dma_start(out_d[t * 128:(t + 1) * 128, :], osb[:])
    nc.compile()
    return nc


# ------------------------------------------------------------------- driver
def kernel(x_src, x_dst, edge_src, edge_dst, W, attn_l, attn_r, bias):
    shared, per_core, k_tile, nch = _host_prep(
        np.asarray(x_src), np.asarray(x_dst), np.asarray(edge_src),
        np.asarray(edge_dst), np.asarray(W), np.asarray(attn_l),
        np.asarray(attn_r), np.asarray(bias))

    nc = _build_nc(k_tile, nch)

    in_maps = []
    for c in range(NCORES):
        in_maps.append({"xT": shared["xT"], "Wext": shared["Wext"],
                        "bias_rep": shared["bias_rep"],
                        "selT": per_core[c]["selT"],
                        "selD": per_core[c]["selD"],
                        "zidx": per_core[c]["zidx"],
                        "eridx": per_core[c]["eridx"]})

    if os.environ.get("KERNEL_SIM"):
        from concourse.bass_interp import CoreSim
        sim = CoreSim(nc, trace=False)
        for name, arr in in_maps[int(os.environ.get("KERNEL_SIM_CORE", "0"))].items():
            sim.tensor(name)[:] = arr
        sim.simulate()
        out = np.array(sim.tensor("out"))
        return np.concatenate([out[:DPC]] * NCORES, 0)  # core-0 slice only

    from concourse.bass_utils import run_bass_kernel_spmd
    res = run_bass_kernel_spmd(nc, in_maps, core_ids=list(range(NCORES)),
                               trace=bool(os.environ.get("KERNEL_TRACE")))
    global LAST_RESULTS
    LAST_RESULTS = res
    return np.concatenate([r["out"][:DPC] for r in res.results], 0)


LAST_RESULTS = None


# revision 17
# speedup vs baseline: 1.9163x; 1.2779x over previous
"""GAT-style bipartite graph attention layer (nn_BiGraphContrastLayer) on 8 trn2 cores.

Strategy (dst-sharded SPMD, one shared program):
  - Every core computes zel = x @ [W | W@Al | W@Ar] for all N nodes (replicated;
    bf16 matmul, fp32 accum) and writes a per-node row table
    zel_tab[n] = [z(512) | el(8) | er(8) | pad] (bf16, 640 elems = 1280B) to DRAM.
  - Each core owns 1250 dst nodes.  Their incoming edges (+ self loops), sorted
    by dst and grouped into 10 dst tiles of 128, are gathered per edge from
    zel_tab via SWDGE dma_gather (src row: 1280B; dst el/er tail: 256B).
  - v = exp(leaky_relu(el_src + er_dst)) per edge/head; messages msg = v * z_src
    (DVE, per-head broadcast); per-dst-tile segment sums via one-hot selection
    matmuls on the PE accumulating in PSUM: out_tile = SelT.T @ msg and
    s_tile = SelT.T @ v.  Final: out/s + bias.
  No inter-core communication; host concatenates the 8 dst slices.
"""
import os

import numpy as np
import ml_dtypes

import concourse.bacc as bacc
import concourse.bass as bass
import concourse.mybir as mybir
import concourse.tile as tile

BF = ml_dtypes.bfloat16
F32 = np.float32

NS, ND, E, DIN, H, DH = 10000, 10000, 320000, 512, 8, 64
NEG = 0.2
NCORES = 8
DPC = ND // NCORES          # 1250 dst nodes per core
N = NS + ND
NPAD = 20480                # node count padded to 160 tiles of 128
ROW = 640                   # zel row elems: z(512) | el(8) | er(8) | pad(112)
NTILES = (DPC + 127) // 128  # 10 dst tiles per core
PANEL = 2048                # phase-1 node panel (16 subtiles of 128)


# ----------------------------------------------------------------- host prep
def _wrap_idx(idx):
    """dma_gather index layout: idx i -> [i % 16, i // 16], replicated 8x."""
    k = len(idx)
    w = np.zeros((16, k // 16), np.int16)
    w[np.arange(k) % 16, np.arange(k) // 16] = idx
    return np.tile(w, (8, 1))


def _host_prep(x_src, x_dst, edge_src, edge_dst, W, attn_l, attn_r, bias):
    x = np.concatenate([x_src, x_dst], 0).astype(F32)       # [N, 512]
    xT = np.zeros((DIN, NPAD), BF)
    xT[:, :N] = x.T
    Al = np.zeros((DIN, H), F32)
    Ar = np.zeros((DIN, H), F32)
    for h in range(H):
        Al[h * DH:(h + 1) * DH, h] = attn_l[h]
        Ar[h * DH:(h + 1) * DH, h] = attn_r[h]
    Wext = np.concatenate([W, W @ Al, W @ Ar], 1).astype(BF)  # [512, 528]
    bias_rep = np.tile(bias[None, :].astype(F32), (128, 1))   # [128, 512]

    # per-(core, dst tile) edge lists, sorted by local dst
    edge_src = edge_src.astype(np.int64)
    edge_dst = edge_dst.astype(np.int64)
    tlists = [[None] * NTILES for _ in range(NCORES)]
    kmax = 0
    for c in range(NCORES):
        d0 = c * DPC
        m = (edge_dst >= d0) & (edge_dst < d0 + DPC)
        es = np.concatenate([edge_src[m],
                             NS + d0 + np.arange(DPC, dtype=np.int64)])
        ed = np.concatenate([edge_dst[m] - d0, np.arange(DPC, dtype=np.int64)])
        order = np.argsort(ed, kind="stable")
        es, ed = es[order], ed[order]
        for t in range(NTILES):
            sel = (ed >= t * 128) & (ed < (t + 1) * 128)
            tlists[c][t] = (es[sel], ed[sel] - t * 128)
            kmax = max(kmax, int(sel.sum()))
    k_tile = ((kmax + 127) // 128) * 128
    nch = k_tile // 128

    per_core = []
    for c in range(NCORES):
        zidx = np.zeros((128, NTILES * k_tile // 16), np.int16)
        selT = np.zeros((128, NTILES * nch * 128), BF)
        selD = np.zeros((128, NTILES * nch * 128), BF)
        for t in range(NTILES):
            es, edl = tlists[c][t]
            k = len(es)
            src = np.zeros(k_tile, np.int64)
            src[:k] = es
            s16 = slice(t * k_tile // 16, (t + 1) * k_tile // 16)
            zidx[:, s16] = _wrap_idx(src)
            dstl = np.full(k_tile, -1, np.int64)
            dstl[:k] = edl
            for ch in range(nch):
                dl = dstl[ch * 128:(ch + 1) * 128]
                sm = np.zeros((128, 128), F32)
                valid = dl >= 0
                sm[np.arange(128)[valid], dl[valid]] = 1.0
                j = (t * nch + ch) * 128
                selT[:, j:j + 128] = sm.astype(BF)
                selD[:, j:j + 128] = sm.T.astype(BF)
        eridx = _wrap_idx(NS + c * DPC + np.arange(NTILES * 128, dtype=np.int64))
        per_core.append(dict(selT=selT, selD=selD, zidx=zidx, eridx=eridx))

    shared = dict(xT=xT, Wext=Wext, bias_rep=bias_rep)
    return shared, per_core, k_tile, nch


# ------------------------------------------------------------- bass program
def _build_nc(k_tile, nch):
    nc = bacc.Bacc("TRN2", target_bir_lowering=False, debug=False)
    dt = mybir.dt

    xT_d = nc.dram_tensor("xT", [DIN, NPAD], dt.bfloat16, kind="ExternalInput")
    W_d = nc.dram_tensor("Wext", [DIN, 528], dt.bfloat16, kind="ExternalInput")
    bias_d = nc.dram_tensor("bias_rep", [128, 512], dt.float32, kind="ExternalInput")
    selT_d = nc.dram_tensor("selT", [128, NTILES * nch * 128], dt.bfloat16,
                            kind="ExternalInput")
    selD_d = nc.dram_tensor("selD", [128, NTILES * nch * 128], dt.bfloat16,
                            kind="ExternalInput")
    zidx_d = nc.dram_tensor("zidx", [128, NTILES * k_tile // 16], dt.int16,
                            kind="ExternalInput")
    eridx_d = nc.dram_tensor("eridx", [128, NTILES * 128 // 16], dt.int16,
                             kind="ExternalInput")
    out_d = nc.dram_tensor("out", [NTILES * 128, 512], dt.float32,
                           kind="ExternalOutput")
    zel_d = nc.dram_tensor("zel_tab", [NPAD, ROW], dt.bfloat16)

    with tile.TileContext(nc) as tc:
        # ---- constants resident in SBUF
        with tc.tile_pool(name="const", bufs=1) as cpool:
            wsb = cpool.tile([128, 4 * 528], dt.bfloat16)
            for k in range(4):
                nc.sync.dma_start(wsb[:, k * 528:(k + 1) * 528],
                                  W_d[k * 128:(k + 1) * 128, :])
            bias_sb = cpool.tile([128, 512], dt.float32)
            nc.sync.dma_start(bias_sb[:], bias_d[:])
            zidx_sb = cpool.tile([128, NTILES * k_tile // 16], dt.int16)
            nc.sync.dma_start(zidx_sb[:], zidx_d[:])
            eridx_sb = cpool.tile([128, NTILES * 128 // 16], dt.int16)
            nc.sync.dma_start(eridx_sb[:], eridx_d[:])

            # ---- phase 1: zel_tab = [x@W | x@Wl | x@Wr] for all nodes
            with (
                tc.tile_pool(name="xp", bufs=2) as xpool,
                tc.tile_pool(name="zel", bufs=3) as zpool,
                tc.tile_pool(name="p1", bufs=2, space="PSUM") as p1pool,
                tc.tile_pool(name="p1b", bufs=2, space="PSUM") as p1bpool,
            ):
                for p in range(NPAD // PANEL):
                    xp = xpool.tile([128, 4 * PANEL], dt.bfloat16)
                    for k in range(4):
                        nc.sync.dma_start(
                            xp[:, k * PANEL:(k + 1) * PANEL],
                            xT_d[k * 128:(k + 1) * 128,
                                 p * PANEL:(p + 1) * PANEL])
                    for m in range(PANEL // 128):
                        zps = p1pool.tile([128, 512], dt.float32, space="PSUM")
                        lps = p1bpool.tile([128, 16], dt.float32, space="PSUM")
                        for k in range(4):
                            lhsT = xp[:, k * PANEL + m * 128:
                                      k * PANEL + (m + 1) * 128]
                            nc.tensor.matmul(zps[:], lhsT,
                                             wsb[:, k * 528:k * 528 + 512],
                                             start=(k == 0), stop=(k == 3))
                            nc.tensor.matmul(lps[:], lhsT,
                                             wsb[:, k * 528 + 512:(k + 1) * 528],
                                             start=(k == 0), stop=(k == 3))
                        zel_sb = zpool.tile([128, ROW], dt.bfloat16)
                        nc.vector.tensor_copy(zel_sb[:, 0:512], zps[:])
                        nc.vector.tensor_copy(zel_sb[:, 512:528], lps[:])
                        nc.gpsimd.memset(zel_sb[:, 528:ROW], 0)
                        row0 = (p * (PANEL // 128) + m) * 128
                        nc.sync.dma_start(zel_d[row0:row0 + 128, :], zel_sb[:])

            # all phase-1 zel_tab writes must land before gathers read it
            tc.strict_bb_all_engine_barrier()

            # ---- phase 2: per dst tile gather + attention + aggregation
            with (
                tc.tile_pool(name="zg", bufs=3) as zgpool,
                tc.tile_pool(name="era", bufs=1) as erapool,
                tc.tile_pool(name="sel", bufs=3) as selpool,
                tc.tile_pool(name="sc", bufs=3) as scpool,
                tc.tile_pool(name="eo", bufs=2) as eopool,
                tc.tile_pool(name="p2", bufs=3, space="PSUM") as p2pool,
                tc.tile_pool(name="p2b", bufs=3, space="PSUM") as p2bpool,
                tc.tile_pool(name="p2c", bufs=2, space="PSUM") as p2cpool,
            ):
                # er (and el) of this core's dst nodes: one small gather
                era = erapool.tile([128, NTILES, 128], dt.bfloat16)
                nc.gpsimd.dma_gather(
                    era[:], zel_d[:, 512:640], eridx_sb[:],
                    num_idxs=NTILES * 128, num_idxs_reg=NTILES * 128,
                    elem_size=128, elem_step=ROW, single_packet=False)

                for t in range(NTILES):
                    i16 = slice(t * k_tile // 16, (t + 1) * k_tile // 16)
                    zg = zgpool.tile([128, nch, ROW], dt.bfloat16)
                    nc.gpsimd.dma_gather(
                        zg[:], zel_d[:], zidx_sb[:, i16],
                        num_idxs=k_tile, num_idxs_reg=k_tile, elem_size=ROW,
                        single_packet=False)
                    sel = selpool.tile([128, nch * 128], dt.bfloat16)
                    nc.sync.dma_start(
                        sel[:], selT_d[:, t * nch * 128:(t + 1) * nch * 128])
                    seld = selpool.tile([128, nch * 128], dt.bfloat16,
                                        tag="seld")
                    nc.sync.dma_start(
                        seld[:], selD_d[:, t * nch * 128:(t + 1) * nch * 128])

                    # er_dst broadcast to edges via Sel matmuls — all chunks
                    # packed into one PSUM bank, then ONE logit add.
                    lt = scpool.tile([128, nch, 8], dt.float32, tag="lt")
                    pe_er = p2cpool.tile([128, nch, 8], dt.float32,
                                         space="PSUM")
                    for ch in range(nch):
                        nc.tensor.matmul(pe_er[:, ch, :],
                                         seld[:, ch * 128:(ch + 1) * 128],
                                         era[:, t, 8:16],
                                         start=True, stop=True,
                                         skip_group_check=True)
                    nc.vector.tensor_tensor(
                        lt[:], zg[:, :, 512:520], pe_er[:],
                        op=mybir.AluOpType.add)
                    nc.vector.scalar_tensor_tensor(
                        lt[:], lt[:], NEG, lt[:],
                        op0=mybir.AluOpType.mult, op1=mybir.AluOpType.max)
                    vt = scpool.tile([128, nch, 8], dt.float32, tag="vt")
                    nc.scalar.activation(vt[:], lt[:],
                                         mybir.ActivationFunctionType.Exp)
                    vb = scpool.tile([128, nch, 8], dt.bfloat16, tag="vb")
                    nc.vector.tensor_copy(vb[:], vt[:])

                    # msg = v * z  (in place over the z part of zg)
                    z4 = zg[:, :, 0:512].rearrange("p c (h d) -> p c h d", d=DH)
                    nc.vector.tensor_tensor(
                        z4, z4, vb[:].to_broadcast([128, nch, 8, DH]),
                        op=mybir.AluOpType.mult)

                    # segment sums on the PE
                    po = p2pool.tile([128, 512], dt.float32, space="PSUM")
                    ps = p2bpool.tile([128, 8], dt.float32, space="PSUM")
                    for ch in range(nch):
                        sl = sel[:, ch * 128:(ch + 1) * 128]
                        nc.tensor.matmul(po[:], sl, zg[:, ch, 0:512],
                                         start=(ch == 0), stop=(ch == nch - 1))
                        nc.tensor.matmul(ps[:], sl, vb[:, ch, :],
                                         start=(ch == 0), stop=(ch == nch - 1))

                    # out = po / s + bias  (eps keeps pad rows finite: 1/eps * 0 = 0)
                    ssb = scpool.tile([128, 8], dt.float32, tag="ssb")
                    nc.vector.tensor_scalar_add(ssb[:], ps[:], 1e-30)
                    nc.vector.reciprocal(ssb[:], ssb[:])
                    osb = eopool.tile([128, 512], dt.float32)
                    o4 = osb[:].rearrange("p (h d) -> p h d", d=DH)
                    nc.vector.tensor_tensor(
                        o4, po[:].rearrange("p (h d) -> p h d", d=DH),
                        ssb[:].to_broadcast([128, 8, DH]),
                        op=mybir.AluOpType.mult)
                    nc.vector.tensor_tensor(osb[:], osb[:], bias_sb[:],
                                            op=mybir.AluOpType.add)
                    nc.sync.dma_start(out_d[t * 128:(t + 1) * 128, :], osb[:])
    nc.compile()
    return nc


# ------------------------------------------------------------------- driver
def kernel(x_src, x_dst, edge_src, edge_dst, W, attn_l, attn_r, bias):
    shared, per_core, k_tile, nch = _host_prep(
        np.asarray(x_src), np.asarray(x_dst), np.asarray(edge_src),
        np.asarray(edge_dst), np.asarray(W), np.asarray(attn_l),
        np.asarray(attn_r), np.asarray(bias))

    nc = _build_nc(k_tile, nch)

    in_maps = []
    for c in range(NCORES):
        in_maps.append({"xT": shared["xT"], "Wext": shared["Wext"],
                        "bias_rep": shared["bias_rep"],
                        "selT": per_core[c]["selT"],
                        "selD": per_core[c]["selD"],
                        "zidx": per_core[c]["zidx"],
                        "eridx": per_core[c]["eridx"]})

    if os.environ.get("KERNEL_SIM"):
        from concourse.bass_interp import CoreSim
        sim = CoreSim(nc, trace=False)
        for name, arr in in_maps[int(os.environ.get("KERNEL_SIM_CORE", "0"))].items():
            sim.tensor(name)[:] = arr
        sim.simulate()
        out = np.array(sim.tensor("out"))
        return np.concatenate([out[:DPC]] * NCORES, 0)  # core-0 slice only

    from concourse.bass_utils import run_bass_kernel_spmd
    res = run_bass_kernel_spmd(nc, in_maps, core_ids=list(range(NCORES)),
                               trace=bool(os.environ.get("KERNEL_TRACE")))
    global LAST_RESULTS
    LAST_RESULTS = res
    return np.concatenate([r["out"][:DPC] for r in res.results], 0)


LAST_RESULTS = None
